# revision 1
# baseline (speedup 1.0000x reference)
"""Attention-LSTM decoder (nn_Decoder) on 8 Trainium2 NeuronCores.

Sharding: data-parallel over batch (8 batches/core), no cross-core
communication. Each core runs the 99-step recurrence for its 8 batches
(transposed [feature-on-partition, batch-on-free] layouts so LSTM gates land
[gate, batch] in PSUM), stores h2/context activations, then runs the big
output projection [792, 256] @ [256, 32000] against a streamed Wout in
float32r (1 cyc/row tensor mode). Host does embedding gather, the layout
transposes, output bias add and reassembly.
"""
import numpy as np

N, TDEC, TENC, HD, KS, VS, V = 64, 100, 512, 512, 128, 128, 32000
NSTEP = TDEC - 1          # 99 decode steps
NC = 8                    # cores
NB = N // NC              # 8 batches per core
CH, NSUB, SUB = 2000, 4, 500   # phase-2 vocab chunking (16 chunks)
NCH = V // CH

_CACHE = {}


def _build_nc(n_steps=NSTEP):
    import concourse.bacc as bacc
    import concourse.mybir as mybir
    from concourse.tile import TileContext
    from bass_rust import add_dep_helper

    f32 = mybir.dt.float32
    f32r = mybir.dt.float32r
    Act = mybir.ActivationFunctionType
    Alu = mybir.AluOpType

    nc = bacc.Bacc("TRN2", target_bir_lowering=False, debug=False)

    xT = nc.dram_tensor("xT", [128, n_steps * 4 * NB], f32, kind="ExternalInput")
    ekT = nc.dram_tensor("ekT", [128, NB * 4 * 128], f32, kind="ExternalInput")
    vals = nc.dram_tensor("vals", [128, NB * 4 * 128], f32, kind="ExternalInput")
    maskT = nc.dram_tensor("maskT", [128, 4 * NB], f32, kind="ExternalInput")
    w1T = nc.dram_tensor("w1T", [128, 9 * 2048], f32, kind="ExternalInput")
    b1B = nc.dram_tensor("b1B", [128, 16 * NB], f32, kind="ExternalInput")
    w2T = nc.dram_tensor("w2T", [128, 5 * 512], f32, kind="ExternalInput")
    b2B = nc.dram_tensor("b2B", [128, 4 * NB], f32, kind="ExternalInput")
    woutT = nc.dram_tensor("woutT", [128, 2 * V], f32r, kind="ExternalInput")
    preds = nc.dram_tensor("preds", [n_steps * NB, V], f32, kind="ExternalOutput")
    attnE = nc.dram_tensor("attnE", [n_steps, 128, 4 * NB], f32, kind="ExternalOutput")
    attnZ = nc.dram_tensor("attnZ", [1, n_steps * NB], f32, kind="ExternalOutput")

    NMT = 7 if n_steps == 99 else (n_steps * NB + 127) // 128  # phase-2 row tiles

    with TileContext(nc) as tc:
        with tc.tile_pool(name="const", bufs=1) as cp:
            w1sb = cp.tile([128, 9 * 2048], f32)
            w2sb = cp.tile([128, 5 * 512], f32)
            eksb = cp.tile([128, NB * 4 * 128], f32)
            vsb = cp.tile([128, NB * 4 * 128], f32)
            xsb = cp.tile([128, n_steps * 4 * NB], f32)
            msb = cp.tile([128, 4 * NB], f32)
            b1sb = cp.tile([128, 16 * NB], f32)
            b2sb = cp.tile([128, 4 * NB], f32)
            onesP = cp.tile([128, 1], f32)
            ones1 = cp.tile([1, 128], f32)
            acts = cp.tile([128, 2 * n_steps * NB], f32r)
            zall = cp.tile([1, n_steps * NB], f32)
            h1 = cp.tile([128, 4 * NB], f32)
            c1 = cp.tile([128, 4 * NB], f32)
            h2 = cp.tile([128, NB], f32)
            c2 = cp.tile([128, NB], f32)
            ctx = cp.tile([128, NB], f32)

            nc.sync.dma_start(w1sb[:], w1T[:])
            nc.sync.dma_start(w2sb[:], w2T[:])
            nc.sync.dma_start(eksb[:], ekT[:])
            nc.sync.dma_start(vsb[:], vals[:])
            nc.sync.dma_start(xsb[:], xT[:])
            nc.sync.dma_start(msb[:], maskT[:])
            nc.sync.dma_start(b1sb[:], b1B[:])
            nc.sync.dma_start(b2sb[:], b2B[:])
            nc.gpsimd.memset(onesP[:], 1.0)
            nc.gpsimd.memset(ones1[:], 1.0)
            nc.gpsimd.memset(h1[:], 0.0)
            nc.gpsimd.memset(c1[:], 0.0)
            nc.gpsimd.memset(h2[:], 0.0)
            nc.gpsimd.memset(c2[:], 0.0)
            nc.gpsimd.memset(ctx[:], 0.0)

            with tc.tile_pool(name="wk", bufs=2) as wk, \
                 tc.tile_pool(name="pgx", bufs=2, space="PSUM") as pgx, \
                 tc.tile_pool(name="pgh", bufs=1, space="PSUM") as pgh, \
                 tc.tile_pool(name="pg2", bufs=1, space="PSUM") as pg2, \
                 tc.tile_pool(name="pe_", bufs=1, space="PSUM") as pe_, \
                 tc.tile_pool(name="pzr", bufs=1, space="PSUM") as pzr, \
                 tc.tile_pool(name="pcx", bufs=1, space="PSUM") as pcx:

                def chain(groups):
                    for i in range(1, len(groups)):
                        add_dep_helper(groups[i][0].ins, groups[i - 1][1].ins,
                                       False, "psum group order")

                for t in range(n_steps):
                    # ---- LSTM1 gates: x part (kc 0-3) and h part (kc 4-8) ----
                    g1x = pgx.tile([128, 16 * NB], f32, tag="g1x")
                    g1h = pgh.tile([128, 16 * NB], f32, tag="g1h")
                    grps = []
                    for mt in range(16):
                        first = last = None
                        for kc in range(4):
                            mm = nc.tensor.matmul(
                                g1x[:, mt * NB:(mt + 1) * NB],
                                w1sb[:, kc * 2048 + mt * 128: kc * 2048 + (mt + 1) * 128],
                                xsb[:, (t * 4 + kc) * NB:(t * 4 + kc + 1) * NB],
                                start=(kc == 0), stop=(kc == 3))
                            first = first or mm
                            last = mm
                        grps.append((first, last))
                    chain(grps)
                    grps = []
                    for mt in range(16):
                        first = last = None
                        for kc in range(4, 9):
                            rhs = ctx[:] if kc == 4 else h1[:, (kc - 5) * NB:(kc - 4) * NB]
                            mm = nc.tensor.matmul(
                                g1h[:, mt * NB:(mt + 1) * NB],
                                w1sb[:, kc * 2048 + mt * 128: kc * 2048 + (mt + 1) * 128],
                                rhs, start=(kc == 4), stop=(kc == 8))
                            first = first or mm
                            last = mm
                        grps.append((first, last))
                    chain(grps)

                    gtmp = wk.tile([128, 16 * NB], f32, tag="gtmp")
                    gsb = wk.tile([128, 16 * NB], f32, tag="gsb")
                    nc.vector.tensor_add(gtmp[:], g1x[:], b1sb[:])
                    nc.vector.tensor_add(gsb[:], g1h[:], gtmp[:])

                    # ---- LSTM1 cell (sigmoid via tanh: sig(x)=0.5*tanh(x/2)+0.5)
                    ti = wk.tile([128, 4 * NB], f32, tag="ti")
                    tf = wk.tile([128, 4 * NB], f32, tag="tf")
                    tg = wk.tile([128, 4 * NB], f32, tag="tg")
                    to = wk.tile([128, 4 * NB], f32, tag="to")
                    S = 4 * NB
                    nc.scalar.activation(ti[:], gsb[:, 0 * S:1 * S], Act.Tanh, scale=0.5)
                    nc.scalar.activation(tf[:], gsb[:, 1 * S:2 * S], Act.Tanh, scale=0.5)
                    nc.scalar.activation(tg[:], gsb[:, 2 * S:3 * S], Act.Tanh)
                    nc.scalar.activation(to[:], gsb[:, 3 * S:4 * S], Act.Tanh, scale=0.5)
                    fi = wk.tile([128, S], f32, tag="fi")
                    ff = wk.tile([128, S], f32, tag="ff")
                    fo = wk.tile([128, S], f32, tag="fo")
                    nc.vector.tensor_scalar(fi[:], ti[:], 0.5, 0.5, Alu.mult, Alu.add)
                    nc.vector.tensor_scalar(ff[:], tf[:], 0.5, 0.5, Alu.mult, Alu.add)
                    nc.vector.tensor_scalar(fo[:], to[:], 0.5, 0.5, Alu.mult, Alu.add)
                    u1 = wk.tile([128, S], f32, tag="u1")
                    u2 = wk.tile([128, S], f32, tag="u2")
                    nc.vector.tensor_mul(u1[:], ff[:], c1[:])
                    nc.vector.tensor_mul(u2[:], fi[:], tg[:])
                    nc.vector.tensor_add(c1[:], u1[:], u2[:])
                    th1 = wk.tile([128, S], f32, tag="th1")
                    nc.scalar.activation(th1[:], c1[:], Act.Tanh)
                    nc.vector.tensor_mul(h1[:], fo[:], th1[:])

                    # ---- LSTM2 ----
                    g2 = pg2.tile([128, 4 * NB], f32, tag="g2")
                    grps = []
                    for mt in range(4):
                        first = last = None
                        for kc in range(5):
                            rhs = h1[:, kc * NB:(kc + 1) * NB] if kc < 4 else h2[:]
                            mm = nc.tensor.matmul(
                                g2[:, mt * NB:(mt + 1) * NB],
                                w2sb[:, kc * 512 + mt * 128: kc * 512 + (mt + 1) * 128],
                                rhs, start=(kc == 0), stop=(kc == 4))
                            first = first or mm
                            last = mm
                        grps.append((first, last))
                    chain(grps)
                    gs2 = wk.tile([128, 4 * NB], f32, tag="gs2")
                    nc.vector.tensor_add(gs2[:], g2[:], b2sb[:])
                    ti2 = wk.tile([128, NB], f32, tag="ti2")
                    tf2 = wk.tile([128, NB], f32, tag="tf2")
                    tg2 = wk.tile([128, NB], f32, tag="tg2")
                    to2 = wk.tile([128, NB], f32, tag="to2")
                    nc.scalar.activation(ti2[:], gs2[:, 0 * NB:1 * NB], Act.Tanh, scale=0.5)
                    nc.scalar.activation(tf2[:], gs2[:, 1 * NB:2 * NB], Act.Tanh, scale=0.5)
                    nc.scalar.activation(tg2[:], gs2[:, 2 * NB:3 * NB], Act.Tanh)
                    nc.scalar.activation(to2[:], gs2[:, 3 * NB:4 * NB], Act.Tanh, scale=0.5)
                    fi2 = wk.tile([128, NB], f32, tag="fi2")
                    ff2 = wk.tile([128, NB], f32, tag="ff2")
                    fo2 = wk.tile([128, NB], f32, tag="fo2")
                    nc.vector.tensor_scalar(fi2[:], ti2[:], 0.5, 0.5, Alu.mult, Alu.add)
                    nc.vector.tensor_scalar(ff2[:], tf2[:], 0.5, 0.5, Alu.mult, Alu.add)
                    nc.vector.tensor_scalar(fo2[:], to2[:], 0.5, 0.5, Alu.mult, Alu.add)
                    v1 = wk.tile([128, NB], f32, tag="v1")
                    v2 = wk.tile([128, NB], f32, tag="v2")
                    nc.vector.tensor_mul(v1[:], ff2[:], c2[:])
                    nc.vector.tensor_mul(v2[:], fi2[:], tg2[:])
                    nc.vector.tensor_add(c2[:], v1[:], v2[:])
                    th2 = wk.tile([128, NB], f32, tag="th2")
                    nc.scalar.activation(th2[:], c2[:], Act.Tanh)
                    nc.vector.tensor_mul(h2[:], fo2[:], th2[:])

                    # ---- attention: energy[t_enc, (tc,b)] then exp, mask ----
                    eps = pe_.tile([128, 4 * NB], f32, tag="eps")
                    for b in range(NB):
                        for tcc in range(4):
                            nc.tensor.matmul(
                                eps[:, tcc * NB + b: tcc * NB + b + 1],
                                eksb[:, (b * 4 + tcc) * 128:(b * 4 + tcc + 1) * 128],
                                h2[:, b:b + 1], start=True, stop=True)
                    esb = wk.tile([128, 4 * NB], f32, tag="esb")
                    nc.scalar.activation(esb[:], eps[:], Act.Exp)
                    em = wk.tile([128, 4 * NB], f32, tag="em")
                    nc.vector.tensor_mul(em[:], esb[:], msb[:])
                    # Z = column sums; reciprocal; broadcast to 128 partitions
                    zps = pzr.tile([1, NB], f32, tag="zps")
                    for tcc in range(4):
                        nc.tensor.matmul(zps[:], onesP[:],
                                         em[:, tcc * NB:(tcc + 1) * NB],
                                         start=(tcc == 0), stop=(tcc == 3))
                    nc.vector.reciprocal(zall[:, t * NB:(t + 1) * NB], zps[:])
                    rbps = pzr.tile([128, NB], f32, tag="rbps")
                    nc.tensor.matmul(rbps[:], ones1[:], zall[:, t * NB:(t + 1) * NB],
                                     start=True, stop=True)
                    rb = wk.tile([128, NB], f32, tag="rb")
                    nc.vector.tensor_copy(rb[:], rbps[:])
                    # context (unnormalized) then scale by 1/Z
                    cx = pcx.tile([128, NB], f32, tag="cx")
                    grps = []
                    for b in range(NB):
                        first = last = None
                        for tcc in range(4):
                            mm = nc.tensor.matmul(
                                cx[:, b:b + 1],
                                vsb[:, (b * 4 + tcc) * 128:(b * 4 + tcc + 1) * 128],
                                em[:, tcc * NB + b: tcc * NB + b + 1],
                                start=(tcc == 0), stop=(tcc == 3))
                            first = first or mm
                            last = mm
                        grps.append((first, last))
                    chain(grps)
                    nc.vector.tensor_mul(ctx[:], cx[:], rb[:])

                    # stash activations for the output projection; attn out
                    nc.vector.tensor_copy(acts[:, t * NB:(t + 1) * NB], h2[:])
                    nc.vector.tensor_copy(
                        acts[:, (n_steps + t) * NB:(n_steps + t + 1) * NB], ctx[:])
                    nc.sync.dma_start(attnE[t], em[:])

            nc.sync.dma_start(attnZ[:], zall[:])

            # ---- phase 2: preds[(t,b), :] = [h2|ctx] @ WoutT ----
            with tc.tile_pool(name="wo", bufs=2) as wo, \
                 tc.tile_pool(name="stg", bufs=3) as stg, \
                 tc.tile_pool(name="p2", bufs=4, space="PSUM") as p2:
                for ch in range(NCH):
                    wt = wo.tile([128, 2 * CH], f32r, tag="wt")
                    nc.sync.dma_start(wt[:, 0:CH], woutT[:, ch * CH:(ch + 1) * CH])
                    nc.sync.dma_start(wt[:, CH:2 * CH],
                                      woutT[:, V + ch * CH: V + (ch + 1) * CH])
                    for mt in range(NMT):
                        rows = min(128, n_steps * NB - mt * 128)
                        st = stg.tile([128, CH], f32, tag="st")
                        for sub in range(NSUB):
                            ps = p2.tile([128, SUB], f32, tag="ps")
                            for kc in range(2):
                                nc.tensor.matmul(
                                    ps[:rows, :],
                                    acts[:, kc * n_steps * NB + mt * 128:
                                         kc * n_steps * NB + mt * 128 + rows],
                                    wt[:, kc * CH + sub * SUB: kc * CH + (sub + 1) * SUB],
                                    start=(kc == 0), stop=(kc == 1))
                            dst = st[:rows, sub * SUB:(sub + 1) * SUB]
                            if sub % 2 == 0:
                                nc.vector.tensor_copy(dst, ps[:rows, :])
                            else:
                                nc.scalar.copy(dst, ps[:rows, :])
                        nc.sync.dma_start(
                            preds[mt * 128: mt * 128 + rows, ch * CH:(ch + 1) * CH],
                            st[:rows, :])

    nc.compile()
    return nc


def _get_nc(n_steps=NSTEP):
    if n_steps not in _CACHE:
        _CACHE[n_steps] = _build_nc(n_steps)
    return _CACHE[n_steps]


def _prep_core_inputs(inputs, n_steps=NSTEP):
    """Build the 8 per-core input maps (host-side layout transforms)."""
    emb = inputs["embedding"]
    text = inputs["text"]
    enc_key = inputs["enc_key"]
    values = inputs["values"]
    lens = inputs["lens"]
    xs = emb[text[:, :n_steps]]                       # [64, n, 512]
    W1 = np.concatenate([inputs["Wih1"], inputs["Whh1"]], axis=1)   # [2048, 1152]
    W2 = np.concatenate([inputs["Wih2"], inputs["Whh2"]], axis=1)   # [512, 640]
    b1 = inputs["bih1"] + inputs["bhh1"]
    b2 = inputs["bih2"] + inputs["bhh2"]
    w1T = np.ascontiguousarray(
        W1.reshape(2048, 9, 128).transpose(2, 1, 0).reshape(128, 9 * 2048)).astype(np.float32)
    w2T = np.ascontiguousarray(
        W2.reshape(512, 5, 128).transpose(2, 1, 0).reshape(128, 5 * 512)).astype(np.float32)
    b1B = np.ascontiguousarray(
        np.repeat(b1.reshape(16, 128).T[:, :, None], NB, axis=2).reshape(128, 16 * NB)).astype(np.float32)
    b2B = np.ascontiguousarray(
        np.repeat(b2.reshape(4, 128).T[:, :, None], NB, axis=2).reshape(128, 4 * NB)).astype(np.float32)
    woutT = np.ascontiguousarray(
        inputs["Wout"].reshape(V, 2, 128).transpose(2, 1, 0).reshape(128, 2 * V)).astype(np.float32)
    t_idx = np.arange(TENC)
    in_maps = []
    for c in range(NC):
        bs = slice(c * NB, (c + 1) * NB)
        xT = np.ascontiguousarray(
            xs[bs].reshape(NB, n_steps, 4, 128).transpose(3, 1, 2, 0)
            .reshape(128, n_steps * 4 * NB)).astype(np.float32)
        ekT = np.ascontiguousarray(
            enc_key[bs].reshape(NB, 4, 128, KS).transpose(3, 0, 1, 2)
            .reshape(128, NB * 4 * 128)).astype(np.float32)
        vl = np.ascontiguousarray(
            values[bs].reshape(NB, 4, 128, VS).transpose(2, 0, 1, 3)
            .reshape(128, NB * 4 * 128)).astype(np.float32)
        mk = (t_idx[None, :] < lens[bs][:, None]).astype(np.float32)  # [NB, 512]
        maskT = np.ascontiguousarray(
            mk.reshape(NB, 4, 128).transpose(2, 1, 0).reshape(128, 4 * NB))
        in_maps.append(dict(xT=xT, ekT=ekT, vals=vl, maskT=maskT, w1T=w1T,
                            b1B=b1B, w2T=w2T, b2B=b2B, woutT=woutT))
    return in_maps


def kernel(**inputs):
    from concourse.bass_utils import run_bass_kernel_spmd
    n_steps = NSTEP
    nc = _get_nc(n_steps)
    in_maps = _prep_core_inputs(inputs, n_steps)
    res = run_bass_kernel_spmd(nc, in_maps, list(range(NC)), trace=False)
    bout = inputs["bout"].astype(np.float32)
    predictions = np.empty((N, n_steps, V), np.float32)
    attention = np.empty((N, n_steps, TENC), np.float32)
    for c in range(NC):
        r = res.results[c]
        p = r["preds"].reshape(n_steps, NB, V).transpose(1, 0, 2)
        predictions[c * NB:(c + 1) * NB] = p + bout[None, None, :]
        em = r["attnE"].reshape(n_steps, 128, 4, NB)
        z = r["attnZ"].reshape(n_steps, NB)
        at = em.transpose(3, 0, 2, 1).reshape(NB, n_steps, TENC)
        attention[c * NB:(c + 1) * NB] = at * z.T[:, :, None]
    return predictions, attention


# revision 3
# speedup vs baseline: 29983.0693x; 29983.0693x over previous
"""Attention-LSTM decoder (nn_Decoder) on 8 Trainium2 NeuronCores.

Sharding: data-parallel over batch (8 batches/core), no cross-core
communication. Each core runs the 99-step recurrence for its 8 batches
(transposed [feature-on-partition, batch-on-free] layouts so LSTM gates land
[gate, batch] in PSUM), stores h2/context activations, then runs the big
output projection [792, 256] @ [256, 32000] against a streamed Wout in
float32r (1 cyc/row tensor mode). Host does embedding gather, the layout
transposes, output bias add and reassembly.
"""
import numpy as np

N, TDEC, TENC, HD, KS, VS, V = 64, 100, 512, 512, 128, 128, 32000
NSTEP = TDEC - 1          # 99 decode steps
NC = 8                    # cores
NB = N // NC              # 8 batches per core
CH, NSUB, SUB = 2000, 4, 500   # phase-2 vocab chunking (16 chunks)
NCH = V // CH

_CACHE = {}


def _build_nc(n_steps=NSTEP, p1_bf16=False):
    import concourse.bacc as bacc
    import concourse.mybir as mybir
    from concourse.tile import TileContext
    from bass_rust import add_dep_helper

    f32 = mybir.dt.float32
    f32r = mybir.dt.float32r
    fin = mybir.dt.bfloat16 if p1_bf16 else f32
    Act = mybir.ActivationFunctionType
    Alu = mybir.AluOpType

    nc = bacc.Bacc("TRN2", target_bir_lowering=False, debug=False)

    xT = nc.dram_tensor("xT", [128, n_steps * 4 * NB], fin, kind="ExternalInput")
    ekT = nc.dram_tensor("ekT", [128, NB * 4 * 128], fin, kind="ExternalInput")
    vals = nc.dram_tensor("vals", [128, NB * 4 * 128], fin, kind="ExternalInput")
    maskT = nc.dram_tensor("maskT", [128, 4 * NB], f32, kind="ExternalInput")
    w1T = nc.dram_tensor("w1T", [128, 9 * 2048], fin, kind="ExternalInput")
    b1B = nc.dram_tensor("b1B", [128, 16 * NB], f32, kind="ExternalInput")
    w2T = nc.dram_tensor("w2T", [128, 5 * 512], fin, kind="ExternalInput")
    b2B = nc.dram_tensor("b2B", [128, 4 * NB], f32, kind="ExternalInput")
    woutT = nc.dram_tensor("woutT", [128, 2 * V], f32r, kind="ExternalInput")
    preds = nc.dram_tensor("preds", [n_steps * NB, V], f32, kind="ExternalOutput")
    attnE = nc.dram_tensor("attnE", [n_steps, 128, 4 * NB], f32, kind="ExternalOutput")
    attnZ = nc.dram_tensor("attnZ", [1, n_steps * NB], f32, kind="ExternalOutput")

    NMT = 7 if n_steps == 99 else (n_steps * NB + 127) // 128  # phase-2 row tiles

    with TileContext(nc) as tc:
        with tc.tile_pool(name="const", bufs=1) as cp:
            w1sb = cp.tile([128, 9 * 2048], fin)
            w2sb = cp.tile([128, 5 * 512], fin)
            eksb = cp.tile([128, NB * 4 * 128], fin)
            vsb = cp.tile([128, NB * 4 * 128], fin)
            xsb = cp.tile([128, n_steps * 4 * NB], fin)
            msb = cp.tile([128, 4 * NB], f32)
            b1sb = cp.tile([128, 16 * NB], f32)
            b2sb = cp.tile([128, 4 * NB], f32)
            onesP = cp.tile([128, 1], f32)
            ones1 = cp.tile([1, 128], f32)
            acts = cp.tile([128, 2 * n_steps * NB], f32r)
            zall = cp.tile([1, n_steps * NB], f32)
            h1 = cp.tile([128, 4 * NB], f32)
            c1 = cp.tile([128, 4 * NB], f32)
            h2 = cp.tile([128, NB], f32)
            c2 = cp.tile([128, NB], f32)
            ctx = cp.tile([128, NB], f32)
            h1b = cp.tile([128, 4 * NB], fin)
            h2b = cp.tile([128, NB], fin)
            ctxb = cp.tile([128, NB], fin)

            nc.sync.dma_start(w1sb[:], w1T[:])
            nc.sync.dma_start(w2sb[:], w2T[:])
            nc.sync.dma_start(eksb[:], ekT[:])
            nc.sync.dma_start(vsb[:], vals[:])
            nc.sync.dma_start(xsb[:], xT[:])
            nc.sync.dma_start(msb[:], maskT[:])
            nc.sync.dma_start(b1sb[:], b1B[:])
            nc.sync.dma_start(b2sb[:], b2B[:])
            nc.gpsimd.memset(onesP[:], 1.0)
            nc.gpsimd.memset(ones1[:], 1.0)
            nc.gpsimd.memset(h1[:], 0.0)
            nc.gpsimd.memset(c1[:], 0.0)
            nc.gpsimd.memset(h2[:], 0.0)
            nc.gpsimd.memset(c2[:], 0.0)
            nc.gpsimd.memset(ctx[:], 0.0)
            nc.gpsimd.memset(h1b[:], 0.0)
            nc.gpsimd.memset(h2b[:], 0.0)
            nc.gpsimd.memset(ctxb[:], 0.0)

            with tc.tile_pool(name="wk", bufs=2) as wk, \
                 tc.tile_pool(name="pgx", bufs=2, space="PSUM") as pgx, \
                 tc.tile_pool(name="pgh", bufs=1, space="PSUM") as pgh, \
                 tc.tile_pool(name="pg2", bufs=1, space="PSUM") as pg2, \
                 tc.tile_pool(name="pe_", bufs=1, space="PSUM") as pe_, \
                 tc.tile_pool(name="pzr", bufs=1, space="PSUM") as pzr, \
                 tc.tile_pool(name="pcx", bufs=1, space="PSUM") as pcx:

                def chain(groups):
                    for i in range(1, len(groups)):
                        add_dep_helper(groups[i][0].ins, groups[i - 1][1].ins,
                                       False, "psum group order")

                for t in range(n_steps):
                    # ---- LSTM1 gates: x part (kc 0-3) and h part (kc 4-8) ----
                    g1x = pgx.tile([128, 16 * NB], f32, tag="g1x")
                    g1h = pgh.tile([128, 16 * NB], f32, tag="g1h")
                    grps = []
                    for mt in range(16):
                        first = last = None
                        for kc in range(4):
                            mm = nc.tensor.matmul(
                                g1x[:, mt * NB:(mt + 1) * NB],
                                w1sb[:, kc * 2048 + mt * 128: kc * 2048 + (mt + 1) * 128],
                                xsb[:, (t * 4 + kc) * NB:(t * 4 + kc + 1) * NB],
                                start=(kc == 0), stop=(kc == 3))
                            first = first or mm
                            last = mm
                        grps.append((first, last))
                    chain(grps)
                    grps = []
                    for mt in range(16):
                        first = last = None
                        for kc in range(4, 9):
                            rhs = ctxb[:] if kc == 4 else h1b[:, (kc - 5) * NB:(kc - 4) * NB]
                            mm = nc.tensor.matmul(
                                g1h[:, mt * NB:(mt + 1) * NB],
                                w1sb[:, kc * 2048 + mt * 128: kc * 2048 + (mt + 1) * 128],
                                rhs, start=(kc == 4), stop=(kc == 8))
                            first = first or mm
                            last = mm
                        grps.append((first, last))
                    chain(grps)

                    gtmp = wk.tile([128, 16 * NB], f32, tag="gtmp")
                    gsb = wk.tile([128, 16 * NB], f32, tag="gsb")
                    nc.vector.tensor_add(gtmp[:], g1x[:], b1sb[:])
                    nc.vector.tensor_add(gsb[:], g1h[:], gtmp[:])

                    # ---- LSTM1 cell (sigmoid via tanh: sig(x)=0.5*tanh(x/2)+0.5)
                    ti = wk.tile([128, 4 * NB], f32, tag="ti")
                    tf = wk.tile([128, 4 * NB], f32, tag="tf")
                    tg = wk.tile([128, 4 * NB], f32, tag="tg")
                    to = wk.tile([128, 4 * NB], f32, tag="to")
                    S = 4 * NB
                    nc.scalar.activation(ti[:], gsb[:, 0 * S:1 * S], Act.Tanh, scale=0.5)
                    nc.scalar.activation(tf[:], gsb[:, 1 * S:2 * S], Act.Tanh, scale=0.5)
                    nc.scalar.activation(tg[:], gsb[:, 2 * S:3 * S], Act.Tanh)
                    nc.scalar.activation(to[:], gsb[:, 3 * S:4 * S], Act.Tanh, scale=0.5)
                    fi = wk.tile([128, S], f32, tag="fi")
                    ff = wk.tile([128, S], f32, tag="ff")
                    fo = wk.tile([128, S], f32, tag="fo")
                    nc.vector.tensor_scalar(fi[:], ti[:], 0.5, 0.5, Alu.mult, Alu.add)
                    nc.vector.tensor_scalar(ff[:], tf[:], 0.5, 0.5, Alu.mult, Alu.add)
                    nc.vector.tensor_scalar(fo[:], to[:], 0.5, 0.5, Alu.mult, Alu.add)
                    u1 = wk.tile([128, S], f32, tag="u1")
                    u2 = wk.tile([128, S], f32, tag="u2")
                    nc.vector.tensor_mul(u1[:], ff[:], c1[:])
                    nc.vector.tensor_mul(u2[:], fi[:], tg[:])
                    nc.vector.tensor_add(c1[:], u1[:], u2[:])
                    th1 = wk.tile([128, S], f32, tag="th1")
                    nc.scalar.activation(th1[:], c1[:], Act.Tanh)
                    nc.vector.tensor_mul(h1[:], fo[:], th1[:])
                    nc.vector.tensor_copy(h1b[:], h1[:])

                    # ---- LSTM2 ----
                    g2 = pg2.tile([128, 4 * NB], f32, tag="g2")
                    grps = []
                    for mt in range(4):
                        first = last = None
                        for kc in range(5):
                            rhs = h1b[:, kc * NB:(kc + 1) * NB] if kc < 4 else h2b[:]
                            mm = nc.tensor.matmul(
                                g2[:, mt * NB:(mt + 1) * NB],
                                w2sb[:, kc * 512 + mt * 128: kc * 512 + (mt + 1) * 128],
                                rhs, start=(kc == 0), stop=(kc == 4))
                            first = first or mm
                            last = mm
                        grps.append((first, last))
                    chain(grps)
                    gs2 = wk.tile([128, 4 * NB], f32, tag="gs2")
                    nc.vector.tensor_add(gs2[:], g2[:], b2sb[:])
                    ti2 = wk.tile([128, NB], f32, tag="ti2")
                    tf2 = wk.tile([128, NB], f32, tag="tf2")
                    tg2 = wk.tile([128, NB], f32, tag="tg2")
                    to2 = wk.tile([128, NB], f32, tag="to2")
                    nc.scalar.activation(ti2[:], gs2[:, 0 * NB:1 * NB], Act.Tanh, scale=0.5)
                    nc.scalar.activation(tf2[:], gs2[:, 1 * NB:2 * NB], Act.Tanh, scale=0.5)
                    nc.scalar.activation(tg2[:], gs2[:, 2 * NB:3 * NB], Act.Tanh)
                    nc.scalar.activation(to2[:], gs2[:, 3 * NB:4 * NB], Act.Tanh, scale=0.5)
                    fi2 = wk.tile([128, NB], f32, tag="fi2")
                    ff2 = wk.tile([128, NB], f32, tag="ff2")
                    fo2 = wk.tile([128, NB], f32, tag="fo2")
                    nc.vector.tensor_scalar(fi2[:], ti2[:], 0.5, 0.5, Alu.mult, Alu.add)
                    nc.vector.tensor_scalar(ff2[:], tf2[:], 0.5, 0.5, Alu.mult, Alu.add)
                    nc.vector.tensor_scalar(fo2[:], to2[:], 0.5, 0.5, Alu.mult, Alu.add)
                    v1 = wk.tile([128, NB], f32, tag="v1")
                    v2 = wk.tile([128, NB], f32, tag="v2")
                    nc.vector.tensor_mul(v1[:], ff2[:], c2[:])
                    nc.vector.tensor_mul(v2[:], fi2[:], tg2[:])
                    nc.vector.tensor_add(c2[:], v1[:], v2[:])
                    th2 = wk.tile([128, NB], f32, tag="th2")
                    nc.scalar.activation(th2[:], c2[:], Act.Tanh)
                    nc.vector.tensor_mul(h2[:], fo2[:], th2[:])
                    nc.vector.tensor_copy(h2b[:], h2[:])

                    # ---- attention: energy[t_enc, (tc,b)] then exp, mask ----
                    eps = pe_.tile([128, 4 * NB], f32, tag="eps")
                    for b in range(NB):
                        for tcc in range(4):
                            nc.tensor.matmul(
                                eps[:, tcc * NB + b: tcc * NB + b + 1],
                                eksb[:, (b * 4 + tcc) * 128:(b * 4 + tcc + 1) * 128],
                                h2b[:, b:b + 1], start=True, stop=True)
                    esb = wk.tile([128, 4 * NB], f32, tag="esb")
                    nc.scalar.activation(esb[:], eps[:], Act.Exp)
                    em = wk.tile([128, 4 * NB], f32, tag="em")
                    nc.vector.tensor_mul(em[:], esb[:], msb[:])
                    emb_ = wk.tile([128, 4 * NB], fin, tag="emb_")
                    nc.vector.tensor_copy(emb_[:], em[:])
                    # Z = column sums; reciprocal; broadcast to 128 partitions
                    zps = pzr.tile([1, NB], f32, tag="zps")
                    for tcc in range(4):
                        nc.tensor.matmul(zps[:], onesP[:],
                                         em[:, tcc * NB:(tcc + 1) * NB],
                                         start=(tcc == 0), stop=(tcc == 3))
                    nc.vector.reciprocal(zall[:, t * NB:(t + 1) * NB], zps[:])
                    rbps = pzr.tile([128, NB], f32, tag="rbps")
                    nc.tensor.matmul(rbps[:], ones1[:], zall[:, t * NB:(t + 1) * NB],
                                     start=True, stop=True)
                    rb = wk.tile([128, NB], f32, tag="rb")
                    nc.vector.tensor_copy(rb[:], rbps[:])
                    # context (unnormalized) then scale by 1/Z
                    cx = pcx.tile([128, NB], f32, tag="cx")
                    grps = []
                    for b in range(NB):
                        first = last = None
                        for tcc in range(4):
                            mm = nc.tensor.matmul(
                                cx[:, b:b + 1],
                                vsb[:, (b * 4 + tcc) * 128:(b * 4 + tcc + 1) * 128],
                                emb_[:, tcc * NB + b: tcc * NB + b + 1],
                                start=(tcc == 0), stop=(tcc == 3))
                            first = first or mm
                            last = mm
                        grps.append((first, last))
                    chain(grps)
                    nc.vector.tensor_mul(ctx[:], cx[:], rb[:])
                    nc.vector.tensor_copy(ctxb[:], ctx[:])

                    # stash activations for the output projection; attn out
                    nc.vector.tensor_copy(acts[:, t * NB:(t + 1) * NB], h2[:])
                    nc.vector.tensor_copy(
                        acts[:, (n_steps + t) * NB:(n_steps + t + 1) * NB], ctx[:])
                    nc.sync.dma_start(attnE[t], em[:])

            nc.sync.dma_start(attnZ[:], zall[:])

            # ---- phase 2: preds[(t,b), :] = [h2|ctx] @ WoutT ----
            with tc.tile_pool(name="wo", bufs=2) as wo, \
                 tc.tile_pool(name="stg", bufs=3) as stg, \
                 tc.tile_pool(name="p2", bufs=4, space="PSUM") as p2:
                for ch in range(NCH):
                    wt = wo.tile([128, 2 * CH], f32r, tag="wt")
                    nc.sync.dma_start(wt[:, 0:CH], woutT[:, ch * CH:(ch + 1) * CH])
                    nc.sync.dma_start(wt[:, CH:2 * CH],
                                      woutT[:, V + ch * CH: V + (ch + 1) * CH])
                    for mt in range(NMT):
                        rows = min(128, n_steps * NB - mt * 128)
                        st = stg.tile([128, CH], f32, tag="st")
                        for sub in range(NSUB):
                            ps = p2.tile([128, SUB], f32, tag="ps")
                            for kc in range(2):
                                nc.tensor.matmul(
                                    ps[:rows, :],
                                    acts[:, kc * n_steps * NB + mt * 128:
                                         kc * n_steps * NB + mt * 128 + rows],
                                    wt[:, kc * CH + sub * SUB: kc * CH + (sub + 1) * SUB],
                                    start=(kc == 0), stop=(kc == 1))
                            dst = st[:rows, sub * SUB:(sub + 1) * SUB]
                            if sub % 2 == 0:
                                nc.vector.tensor_copy(dst, ps[:rows, :])
                            else:
                                nc.scalar.copy(dst, ps[:rows, :])
                        nc.sync.dma_start(
                            preds[mt * 128: mt * 128 + rows, ch * CH:(ch + 1) * CH],
                            st[:rows, :])

    nc.compile()
    return nc


def _get_nc(n_steps=NSTEP):
    if n_steps not in _CACHE:
        _CACHE[n_steps] = _build_nc(n_steps)
    return _CACHE[n_steps]


def _prep_core_inputs(inputs, n_steps=NSTEP, p1_bf16=False):
    """Build the 8 per-core input maps (host-side layout transforms)."""
    emb = inputs["embedding"]
    text = inputs["text"]
    enc_key = inputs["enc_key"]
    values = inputs["values"]
    lens = inputs["lens"]
    xs = emb[text[:, :n_steps]]                       # [64, n, 512]
    W1 = np.concatenate([inputs["Wih1"], inputs["Whh1"]], axis=1)   # [2048, 1152]
    W2 = np.concatenate([inputs["Wih2"], inputs["Whh2"]], axis=1)   # [512, 640]
    b1 = inputs["bih1"] + inputs["bhh1"]
    b2 = inputs["bih2"] + inputs["bhh2"]
    w1T = np.ascontiguousarray(
        W1.reshape(2048, 9, 128).transpose(2, 1, 0).reshape(128, 9 * 2048)).astype(np.float32)
    w2T = np.ascontiguousarray(
        W2.reshape(512, 5, 128).transpose(2, 1, 0).reshape(128, 5 * 512)).astype(np.float32)
    b1B = np.ascontiguousarray(
        np.repeat(b1.reshape(16, 128).T[:, :, None], NB, axis=2).reshape(128, 16 * NB)).astype(np.float32)
    b2B = np.ascontiguousarray(
        np.repeat(b2.reshape(4, 128).T[:, :, None], NB, axis=2).reshape(128, 4 * NB)).astype(np.float32)
    woutT = np.ascontiguousarray(
        inputs["Wout"].reshape(V, 2, 128).transpose(2, 1, 0).reshape(128, 2 * V)).astype(np.float32)
    t_idx = np.arange(TENC)
    import ml_dtypes
    bf16 = ml_dtypes.bfloat16
    if p1_bf16:
        w1T = w1T.astype(bf16)
        w2T = w2T.astype(bf16)
    in_maps = []
    for c in range(NC):
        bs = slice(c * NB, (c + 1) * NB)
        xT = np.ascontiguousarray(
            xs[bs].reshape(NB, n_steps, 4, 128).transpose(3, 1, 2, 0)
            .reshape(128, n_steps * 4 * NB)).astype(bf16 if p1_bf16 else np.float32)
        ekT = np.ascontiguousarray(
            enc_key[bs].reshape(NB, 4, 128, KS).transpose(3, 0, 1, 2)
            .reshape(128, NB * 4 * 128)).astype(bf16 if p1_bf16 else np.float32)
        vl = np.ascontiguousarray(
            values[bs].reshape(NB, 4, 128, VS).transpose(2, 0, 1, 3)
            .reshape(128, NB * 4 * 128)).astype(bf16 if p1_bf16 else np.float32)
        mk = (t_idx[None, :] < lens[bs][:, None]).astype(np.float32)  # [NB, 512]
        maskT = np.ascontiguousarray(
            mk.reshape(NB, 4, 128).transpose(2, 1, 0).reshape(128, 4 * NB))
        in_maps.append(dict(xT=xT, ekT=ekT, vals=vl, maskT=maskT, w1T=w1T,
                            b1B=b1B, w2T=w2T, b2B=b2B, woutT=woutT))
    return in_maps


def kernel(**inputs):
    from concourse.bass_utils import run_bass_kernel_spmd
    n_steps = NSTEP
    nc = _get_nc(n_steps)
    in_maps = _prep_core_inputs(inputs, n_steps)
    res = run_bass_kernel_spmd(nc, in_maps, list(range(NC)), trace=False)
    bout = inputs["bout"].astype(np.float32)
    predictions = np.empty((N, n_steps, V), np.float32)
    attention = np.empty((N, n_steps, TENC), np.float32)
    for c in range(NC):
        r = res.results[c]
        p = r["preds"].reshape(n_steps, NB, V).transpose(1, 0, 2)
        predictions[c * NB:(c + 1) * NB] = p + bout[None, None, :]
        em = r["attnE"].reshape(n_steps, 128, 4, NB)
        z = r["attnZ"].reshape(n_steps, NB)
        at = em.transpose(3, 0, 2, 1).reshape(NB, n_steps, TENC)
        attention[c * NB:(c + 1) * NB] = at * z.T[:, :, None]
    return predictions, attention
